# revision 29
# baseline (speedup 1.0000x reference)
"""Trainium2 Bass kernel for nn_BreakthroughSNN (predictive-coding SNN LM).

Strategy (v2):
  - Replicate the sequential 128-step SNN recurrence on all 8 cores (per-step
    tensors are tiny: B=16, D=512); shard the vocab dim of the output
    projection (V=32000 -> 4000 per core). No collectives.
  - All per-step activations live in "T-layout" SBUF tiles [128, 64]:
    partition p, free index c*16+b  <->  element (d = c*128+p, b).
  - All matmuls run WEIGHT-STATIONARY in bf16: out.T chunks [128(dout),16(b)]
    accumulate over 4 k-chunks with the weight [128,128] chunk as the
    stationary operand and the (transposed) activation [128,16] as the
    moving operand.  Outputs land directly in T-layout PSUM -- no
    N-layout intermediates, no PE transposes, no fp32 double-pass.
  - err = LN(relu(bu - pred)) feeding err @ inf_W.T is folded:
      err@W.T = istd * ((r*g)@W.T) - mu*istd*(g@W.T) + (b@W.T + inf_b)
    with r = relu(bu - pred); g folded into the weights on the host;
    G2 = g@W.T and B2 = b@W.T + inf_b host-precomputed.  LN1 stats run
    concurrently with the matmul.
  - Top-down: bu=0 => err = ne_b exactly, so the TD inf-matmul input is the
    constant B2; only gen matmuls remain in the TD chain.  The j=L-1 TD gen
    matmul (td = states[L-1]) doubles as next step's xgen[L-1].  For j<L-1
    the TD matmul and the xgen matmul share one weight pass via an
    interleaved moving tile [128, (k,src,b)].
  - relu(relu(s) - p) = relu(s - p) for p in {0,1} removes the bottom-up relu.
  - Layer-stacked LIF updates: the three mem_gen decay/spike/reset updates at
    step start, the three mem_inf "base" updates, and the three TD mem_inf
    updates each run as single [128, 192] DVE ops.
  - LayerNorm partition-reductions via ones-matmul on PE (ones pre-scaled by
    1/D); per-batch stats broadcast back across partitions via a ones[1,128]
    bcast-matmul; sqrt+reciprocal fused into one Rsqrt activation.
"""

import sys

sys.path.insert(0, "/opt/trn_rl_repo")

import numpy as np
import ml_dtypes

import concourse.bass as bass
import concourse.bacc as bacc
from concourse import mybir
from concourse.bass_utils import run_bass_kernel_spmd
from concourse.tile import TileContext

F32 = mybir.dt.float32
BF16 = mybir.dt.bfloat16
Alu = mybir.AluOpType
Act = mybir.ActivationFunctionType
BFNP = ml_dtypes.bfloat16

B, S, V, D, L = 16, 128, 32000, 512, 3
NCORES = 8
VS = V // NCORES  # 4000 vocab rows per core
DECAY = float(np.exp(-1.0 / 2.0))
THR = 1.0
EPS = 1e-5
C = D // 128  # 4 d-chunks
W64 = C * B  # 64: free width of a T-layout tile
STK = L * W64  # 192: layer-stacked tile width


def _wT_kmajor(W, dt=np.float32):
    """W: (D_out, D_in) -> SBUF layout [128, C*D_out], k-chunk-major.

    slice [:, k*D_out:(k+1)*D_out] is W.T[k*128:(k+1)*128, :]."""
    Dout, Din = W.shape
    return (
        np.ascontiguousarray(W.T)
        .reshape(Din // 128, 128, Dout)
        .transpose(1, 0, 2)
        .reshape(128, (Din // 128) * Dout)
        .astype(dt)
    )


def _vec_T(v):
    """v: (D,) -> T-layout tile [128, 64] (b-independent broadcast)."""
    t = v.reshape(C, 128).T  # [128, C]
    return np.ascontiguousarray(
        np.broadcast_to(t[:, :, None], (128, C, B)).reshape(128, W64)
    ).astype(np.float32)


def _tok_T(tok, dt):
    """tok: (B,S,D) -> c-major [128, C*S*16]: [p, c*S*16 + t*16 + b]."""
    a = tok.transpose(2, 1, 0)  # [D, S, B]
    a = a.reshape(C, 128, S, B).transpose(1, 0, 2, 3)  # [128, C, S, B]
    return np.ascontiguousarray(a.reshape(128, C * S * B)).astype(dt)


def _build(nS):
    """Build the Bass program for nS recurrence steps."""
    nc = bacc.Bacc(None, target_bir_lowering=False)

    # ---- DRAM parameters ----
    d_tok = nc.declare_dram_parameter("tok_t", [128, nS * W64], BF16, isOutput=False)
    d_encw = nc.declare_dram_parameter("encw_t", [128, C * D], BF16, isOutput=False)
    d_genw = nc.declare_dram_parameter("genw_t", [L, 128, C * D], BF16, isOutput=False)
    d_infw = nc.declare_dram_parameter("infw_t", [L, 128, C * D], BF16, isOutput=False)
    d_outw = nc.declare_dram_parameter("outw_t", [128, C * VS], BF16, isOutput=False)
    # small constants, stacked T-layout tiles along free dim [128, 15*64]:
    # 0..2 genb_T, 3..5 B2_T, 6..8 G2_T, 9..11 nsg_T, 12..14 nsb_T
    d_ctile = nc.declare_dram_parameter("ctiles", [128, 15 * W64], F32, isOutput=False)
    d_encb = nc.declare_dram_parameter("encb_c", [128, C], F32, isOutput=False)
    d_onesb = nc.declare_dram_parameter("onesb", [128, 128], BF16, isOutput=False)
    d_logits = nc.declare_dram_parameter("logits", [nS * B, VS], BF16, isOutput=True)

    with TileContext(nc) as tc:
        with (
            tc.tile_pool(name="const", bufs=1) as cpool,
            tc.tile_pool(name="state", bufs=1) as spool,
            tc.tile_pool(name="work", bufs=3) as wpool,
            tc.tile_pool(name="stat", bufs=4) as stpool,
            tc.tile_pool(name="fin", bufs=4) as fpool,
            tc.tile_pool(name="owq", bufs=2) as owq,
            tc.tile_pool(name="psP", bufs=2, space="PSUM") as psP,
            tc.tile_pool(name="psM", bufs=3, space="PSUM") as psM,
            tc.tile_pool(name="psS", bufs=3, space="PSUM") as psS,
        ):
            # ---- load constants / weights into SBUF ----
            encw = cpool.tile([128, C * D], BF16, name="encw")
            nc.sync.dma_start(out=encw, in_=d_encw[:, :])
            genw = [cpool.tile([128, C * D], BF16, tag=f"genw{j}", name=f"genw{j}") for j in range(L)]
            infw = [cpool.tile([128, C * D], BF16, tag=f"infw{j}", name=f"infw{j}") for j in range(L)]
            for j in range(L):
                nc.sync.dma_start(out=genw[j], in_=d_genw[j])
                nc.sync.dma_start(out=infw[j], in_=d_infw[j])

            ct = cpool.tile([128, 15 * W64], F32, name="ct")
            nc.sync.dma_start(out=ct, in_=d_ctile[:, :])
            _cs = lambda i: ct[:, i * W64 : (i + 1) * W64]
            genbS = ct[:, 0:STK]  # stacked genb
            genbT = [_cs(j) for j in range(3)]
            B2S = ct[:, STK : 2 * STK]  # stacked B2
            G2T = [_cs(6 + j) for j in range(3)]
            nsgT = [_cs(9 + j) for j in range(3)]
            nsbT = [_cs(12 + j) for j in range(3)]

            encb = cpool.tile([128, C], F32, name="encb")
            nc.sync.dma_start(out=encb, in_=d_encb[:, :])

            onesb = cpool.tile([128, 128], BF16, name="onesb")
            nc.sync.dma_start(out=onesb, in_=d_onesb[:, :])

            epst = cpool.tile([128, 1], F32, name="epst")
            nc.vector.memset(epst, EPS)

            # tok tile doubles as td history (tok fully consumed in prologue)
            toktd = cpool.tile([128, nS * W64], BF16, name="toktd")
            nc.sync.dma_start(out=toktd, in_=d_tok[:, :])
            xenc = cpool.tile([128, nS * W64], F32, name="xenc")

            # ---- persistent state tiles ----
            mem_enc = spool.tile([128, W64], F32, tag="mem_enc", name="mem_enc")
            mem_gen = spool.tile([128, STK], F32, tag="mgen", name="mgen")
            mem_inf = spool.tile([128, STK], F32, tag="minf", name="minf")
            states = spool.tile([128, STK], BF16, tag="states", name="states")
            xgen = spool.tile([128, STK], F32, tag="xgen", name="xgen")
            # interleaved (k, src, b) moving tiles doubling as storage for
            # states[0] / states[1] (src=1) and pred_td[1] / pred_td[2] (src=0)
            mvt = [spool.tile([128, 2 * W64], BF16, tag=f"mvt{j}", name=f"mvt{j}")
                   for j in range(2)]
            mvt4 = [m.rearrange("p (k s b) -> p k s b", s=2, b=B) for m in mvt]

            def stview(j):
                """[128, C, B] view of states[j]."""
                if j == L - 1:
                    return states[:, j * W64 : (j + 1) * W64].rearrange(
                        "p (c b) -> p c b", c=C
                    )
                return mvt4[j][:, :, 1, :]

            nc.vector.memset(mem_enc, 0.0)
            nc.vector.memset(mem_gen, 0.0)
            nc.vector.memset(mem_inf, 0.0)
            nc.vector.memset(states, 0.0)
            nc.vector.memset(mvt[0], 0.0)
            nc.vector.memset(mvt[1], 0.0)
            # states0 = 0 -> x_gen(t=0) = gen_b
            nc.vector.tensor_copy(xgen, genbS)

            # PE pre-touch so the first real matmul carries only one
            # DMA-queue wait.
            ptch = psS.tile([1, 1], F32, tag="ps", name="ptch")
            nc.tensor.matmul(ptch, onesb[0:1, 0:1], onesb[0:1, 0:1], start=True, stop=True)

            # ---- prologue: x_enc = tok @ enc_W.T + enc_b, all steps ----
            TB = nS * B  # cols per c-chunk in c-major tok layout
            xenc4 = xenc.rearrange("p (t c b) -> p c t b", c=C, b=B)
            nblk = 4 if nS >= 4 else nS
            tblk = nS // nblk  # steps per block
            fd = tblk * B  # 512 moving cols per block
            for m in range(C):
                for ti in range(nblk):
                    pE = psP.tile([128, 512], F32, tag="pbig", name="pE")
                    for k in range(C):
                        nc.tensor.matmul(
                            pE[:, :fd],
                            encw[:, k * D + m * 128 : k * D + m * 128 + 128],
                            toktd[:, k * TB + ti * fd : k * TB + (ti + 1) * fd],
                            start=(k == 0),
                            stop=(k == C - 1),
                        )
                    nc.vector.tensor_scalar(
                        out=xenc4[:, m, ti * tblk : (ti + 1) * tblk, :],
                        in0=pE[:, :fd].rearrange("p (t b) -> p t b", b=B),
                        scalar1=encb[:, m : m + 1],
                        scalar2=None,
                        op0=Alu.add,
                    )

            # ---- helper closures ----
            def as3(t):
                return t.rearrange("p (c b) -> p c b", c=C)

            def bc(bsrc, lo):
                """[128,16] slice of bsrc broadcast to [128,4,16]."""
                return bsrc[:, None, lo : lo + 16].broadcast_to([128, C, 16])

            def ln_stats(vtile, tag):
                """vtile: [128,128] bf16 with v in cols 0:64 and v^2 in 64:128.
                Returns bsrc [128,32] SBUF: cols 0:16 istd, 16:32 mu*istd,
                identical on every partition (full-ones stationary broadcasts
                the sums to all partitions -- no separate bcast matmul)."""
                pst = psS.tile([128, 32], F32, tag="ps", name="pst")
                v4 = vtile.rearrange("p (g c b) -> p c g b", g=2, c=C)
                for c in range(C):
                    nc.tensor.matmul(
                        pst, onesb, v4[:, c], start=(c == 0), stop=(c == C - 1)
                    )
                mu2 = stpool.tile([128, 16], F32, tag=f"mu2_{tag}", name="mu2")
                nc.scalar.activation(out=mu2, in_=pst[:, 0:16], func=Act.Square)
                bsrc = wpool.tile([128, 32], F32, tag=f"bsrc_{tag}", name="bsrc")
                var = bsrc[:, 16:32]
                nc.vector.tensor_sub(var, pst[:, 16:32], mu2)
                nc.scalar.activation(
                    out=bsrc[:, 0:16], in_=var, func=Act.Abs_reciprocal_sqrt,
                    bias=epst[:, 0:1],
                )
                nc.vector.tensor_mul(bsrc[:, 16:32], pst[:, 0:16], bsrc[:, 0:16])
                return bsrc

            def mm_ws(psum, w_sb, rhs_of_k, ncols):
                """Weight-stationary matmul: psum[:, m*ncols:(m+1)*ncols]
                accumulates W_chunk.T-stationary passes; rhs_of_k(k) gives the
                [128, ncols] moving slice for k-chunk k."""
                for m in range(C):
                    for k in range(C):
                        nc.tensor.matmul(
                            psum[:, m * ncols : (m + 1) * ncols],
                            w_sb[:, k * D + m * 128 : k * D + m * 128 + 128],
                            rhs_of_k(k),
                            start=(k == 0),
                            stop=(k == C - 1),
                        )

            # ---- main recurrence ----
            for t in range(nS):
                # encoder LIF
                nc.vector.scalar_tensor_tensor(
                    out=mem_enc, in0=mem_enc, scalar=DECAY,
                    in1=xenc[:, t * W64 : (t + 1) * W64],
                    op0=Alu.mult, op1=Alu.add,
                )
                bu0 = wpool.tile([128, W64], BF16, tag="bu0", name="bu0")
                nc.vector.tensor_scalar(
                    out=bu0, in0=mem_enc, scalar1=THR, scalar2=None, op0=Alu.is_ge
                )
                nc.vector.scalar_tensor_tensor(
                    out=mem_enc, in0=mem_enc, scalar=THR, in1=mem_enc,
                    op0=Alu.is_lt, op1=Alu.mult,
                )

                # stacked gen LIF: pred_j for all layers at once
                nc.vector.scalar_tensor_tensor(
                    out=mem_gen, in0=mem_gen, scalar=DECAY, in1=xgen,
                    op0=Alu.mult, op1=Alu.add,
                )
                pred = wpool.tile([128, STK], BF16, tag="pred", name="pred")
                nc.vector.tensor_scalar(
                    out=pred, in0=mem_gen, scalar1=THR, scalar2=None, op0=Alu.is_ge
                )
                nc.vector.scalar_tensor_tensor(
                    out=mem_gen, in0=mem_gen, scalar=THR, in1=mem_gen,
                    op0=Alu.is_lt, op1=Alu.mult,
                )
                # stacked decayed mem_inf base: base = mem_inf*dec + B2
                basei = wpool.tile([128, STK], F32, tag="basei", name="basei")
                nc.vector.scalar_tensor_tensor(
                    out=basei, in0=mem_inf, scalar=DECAY, in1=B2S,
                    op0=Alu.mult, op1=Alu.add,
                )
                # stacked TD-phase gen pre-bias: mgp = mem_gen*dec + gen_b
                mgp = wpool.tile([128, STK], F32, tag="mgp", name="mgp")
                nc.vector.scalar_tensor_tensor(
                    out=mgp, in0=mem_gen, scalar=DECAY, in1=genbS,
                    op0=Alu.mult, op1=Alu.add,
                )

                # ---- bottom-up ----
                for j in range(L):
                    jc = slice(j * W64, (j + 1) * W64)
                    # r = relu(bu - pred)  (bu = bu0 or states[j-1]; exact for
                    # j>0 because relu(relu(s)-p) = relu(s-p) for p in {0,1})
                    bu_in = as3(bu0) if j == 0 else stview(j - 1)
                    r2 = wpool.tile([128, 2 * W64], BF16, tag="r2", name="r2")
                    r = r2[:, 0:W64]
                    nc.vector.tensor_tensor(
                        out=as3(r), in0=bu_in, in1=as3(pred[:, jc]), op=Alu.subtract
                    )
                    nc.vector.tensor_scalar(
                        out=r, in0=r, scalar1=0.0, scalar2=None, op0=Alu.max
                    )
                    # square on DVE: stays in-queue behind the relu (no ACT
                    # handoff on the critical path)
                    nc.vector.tensor_mul(r2[:, W64 : 2 * W64], r, r)
                    # stats matmuls first on the PE queue (their tail chain is
                    # the critical path; pI is consumed much later)
                    bs1 = ln_stats(r2, f"l1_{j}")
                    pI = psM.tile([128, W64], F32, tag="pmm", name="pI")
                    mm_ws(pI, infw[j], lambda k: r2[:, k * B : (k + 1) * B], B)

                    # mem_inf' = base - mu*istd*G2 + istd*(folded matmul)
                    gsc = wpool.tile([128, W64], F32, tag="gsc", name="gsc")
                    nc.vector.tensor_mul(as3(gsc), as3(G2T[j]), bc(bs1, 16))
                    nc.vector.tensor_sub(gsc, basei[:, jc], gsc)
                    xsc = wpool.tile([128, W64], F32, tag="xsc", name="xsc")
                    nc.vector.tensor_mul(as3(xsc), as3(pI), bc(bs1, 0))
                    nc.vector.tensor_add(mem_inf[:, jc], xsc, gsc)

                    # state' = LN_ns(state + su); su = spike(mem_inf') fused in
                    w2 = wpool.tile([128, 2 * W64], BF16, tag="w2", name="w2")
                    wv = w2[:, 0:W64]
                    nc.vector.scalar_tensor_tensor(
                        out=as3(wv), in0=as3(mem_inf[:, jc]), scalar=THR,
                        in1=stview(j), op0=Alu.is_ge, op1=Alu.add,
                    )
                    nc.vector.scalar_tensor_tensor(
                        out=mem_inf[:, jc], in0=mem_inf[:, jc], scalar=THR,
                        in1=mem_inf[:, jc], op0=Alu.is_lt, op1=Alu.mult,
                    )
                    nc.vector.tensor_mul(w2[:, W64 : 2 * W64], wv, wv)
                    bs2 = ln_stats(w2, f"l2_{j}")
                    t1 = wpool.tile([128, W64], F32, tag="t1", name="t1")
                    nc.vector.tensor_mul(as3(t1), as3(wv), bc(bs2, 0))
                    nc.vector.tensor_tensor(
                        out=as3(t1), in0=as3(t1), in1=bc(bs2, 16), op=Alu.subtract
                    )
                    nc.vector.tensor_mul(t1, t1, nsgT[j])
                    nc.vector.tensor_tensor(
                        out=stview(j), in0=as3(t1), in1=as3(nsbT[j]), op=Alu.add
                    )

                # ---- top-down ----
                # j = L-1: td = states[L-1]; this matmul doubles as xgen[L-1]
                for j in reversed(range(L)):
                    jc = slice(j * W64, (j + 1) * W64)
                    mg_pre = mgp[:, jc]
                    if j == L - 1:
                        pG = psM.tile([128, W64], F32, tag="pmm", name="pG")
                        mm_ws(pG, genw[j],
                              lambda k: states[:, j * W64 + k * B : j * W64 + (k + 1) * B],
                              B)
                        pG_td = as3(pG)      # [128, m, 16] view for mem_gen
                        pG_xg = as3(pG)
                    else:
                        mv = mvt[j]
                        pG = psM.tile([128, 2 * W64], F32, tag="pmm", name="pG2")
                        mm_ws(pG, genw[j],
                              lambda k: mv[:, 2 * B * k : 2 * B * (k + 1)], 2 * B)
                        pg4 = pG.rearrange("p (m s b) -> p m s b", s=2, b=B)
                        pG_td = pg4[:, :, 0, :]
                        pG_xg = pg4[:, :, 1, :]
                    # mem_gen update + pred_td spike
                    nc.vector.tensor_tensor(
                        out=as3(mem_gen[:, jc]), in0=as3(mg_pre), in1=pG_td,
                        op=Alu.add,
                    )
                    if j > 0:
                        # write pred_td[j] into interleaved moving tile for j-1
                        nc.vector.tensor_scalar(
                            out=mvt4[j - 1][:, :, 0, :], in0=as3(mem_gen[:, jc]),
                            scalar1=THR, scalar2=None, op0=Alu.is_ge,
                        )
                    else:
                        # write td_t (binary) into c-major td history
                        p_out = toktd.rearrange(
                            "p (c t b) -> p c t b", c=C, b=B
                        )[:, :, t, :]
                        nc.vector.tensor_scalar(
                            out=p_out, in0=as3(mem_gen[:, jc]),
                            scalar1=THR, scalar2=None, op0=Alu.is_ge,
                        )
                    nc.vector.scalar_tensor_tensor(
                        out=mem_gen[:, jc], in0=mem_gen[:, jc], scalar=THR,
                        in1=mem_gen[:, jc], op0=Alu.is_lt, op1=Alu.mult,
                    )
                    # xgen for next step (+bias)
                    nc.vector.tensor_tensor(
                        out=as3(xgen[:, jc]), in0=pG_xg, in1=as3(genbT[j]),
                        op=Alu.add,
                    )

                # stacked TD mem_inf update with constant input B2 (spike unused)
                nc.vector.scalar_tensor_tensor(
                    out=mem_inf, in0=mem_inf, scalar=DECAY, in1=B2S,
                    op0=Alu.mult, op1=Alu.add,
                )
                nc.vector.scalar_tensor_tensor(
                    out=mem_inf, in0=mem_inf, scalar=THR, in1=mem_inf,
                    op0=Alu.is_lt, op1=Alu.mult,
                )

            # ---- final projection: logits = td @ out_W_shard.T ----
            n_mb = (nS * B) // 128 if nS * B >= 128 else 1
            t_mb = nS // n_mb  # steps per M-block
            NB = VS // 500  # 8 chunks of 500
            for nb in range(NB):
                outwq = owq.tile([128, C * 500], BF16, tag="outwq", name="outwq")
                nc.sync.dma_start(
                    out=outwq, in_=d_outw[:, nb * C * 500 : (nb + 1) * C * 500]
                )
                for mb in range(n_mb):
                    pf = psP.tile([128, 512], F32, tag="pbig", name="pf")
                    fdp = t_mb * B
                    for k in range(C):
                        nc.tensor.matmul(
                            pf[:fdp, 0:500],
                            toktd[:, k * TB + mb * fdp : k * TB + (mb + 1) * fdp],
                            outwq[:, k * 500 : (k + 1) * 500],
                            start=(k == 0),
                            stop=(k == C - 1),
                        )
                    fo = fpool.tile([128, 500], BF16, tag="fo", name="fo")
                    if mb % 2 == 0:
                        nc.scalar.copy(fo[:fdp], pf[:fdp, 0:500])
                    else:
                        nc.vector.tensor_copy(fo[:fdp], pf[:fdp, 0:500])
                    nc.sync.dma_start(
                        out=d_logits[
                            mb * fdp : (mb + 1) * fdp, nb * 500 : (nb + 1) * 500
                        ],
                        in_=fo[:fdp],
                    )

    return nc


_CACHE = {}
TRACE = False
LAST_RESULTS = None


def _get_program(nS):
    if nS not in _CACHE:
        nc = _build(nS)
        nc.finalize()
        _CACHE[nS] = nc
    return _CACHE[nS]


def kernel(**inputs):
    input_ids = np.asarray(inputs["input_ids"])
    emb = np.asarray(inputs["emb"], np.float32)
    enc_W = np.asarray(inputs["enc_W"], np.float32)
    enc_b = np.asarray(inputs["enc_b"], np.float32)
    gen_W = np.asarray(inputs["gen_W"], np.float32)
    gen_b = np.asarray(inputs["gen_b"], np.float32)
    inf_W = np.asarray(inputs["inf_W"], np.float32)
    inf_b = np.asarray(inputs["inf_b"], np.float32)
    ns_g = np.asarray(inputs["ns_g"], np.float32)
    ns_b = np.asarray(inputs["ns_b"], np.float32)
    ne_g = np.asarray(inputs["ne_g"], np.float32)
    ne_b = np.asarray(inputs["ne_b"], np.float32)
    out_W = np.asarray(inputs["out_W"], np.float32)
    out_b = np.asarray(inputs["out_b"], np.float32)

    nB, nS = input_ids.shape
    assert (nB, nS) == (B, S), (nB, nS)

    tok = emb[input_ids]  # (B, S, D) host gather

    # host-side constant folding
    ctiles = np.zeros((15, 128, W64), np.float32)
    genw_t = np.zeros((L, 128, C * D), BFNP)
    infw_t = np.zeros((L, 128, C * D), BFNP)
    for j in range(L):
        ctiles[j] = _vec_T(gen_b[j])
        B2 = ne_b[j] @ inf_W[j].T + inf_b[j]
        G2 = ne_g[j] @ inf_W[j].T
        ctiles[3 + j] = _vec_T(B2)
        ctiles[6 + j] = _vec_T(G2)
        ctiles[9 + j] = _vec_T(ns_g[j])
        ctiles[12 + j] = _vec_T(ns_b[j])
        genw_t[j] = _wT_kmajor(gen_W[j], BFNP)
        infw_t[j] = _wT_kmajor(inf_W[j] * ne_g[j][None, :], BFNP)
    ctiles_packed = np.ascontiguousarray(
        ctiles.transpose(1, 0, 2).reshape(128, 15 * W64)
    )

    shared = {
        "tok_t": _tok_T(tok, BFNP),
        "encw_t": _wT_kmajor(enc_W, BFNP),
        "genw_t": genw_t,
        "infw_t": infw_t,
        "ctiles": ctiles_packed,
        "encb_c": np.ascontiguousarray(enc_b.reshape(C, 128).T).astype(np.float32),
        "onesb": np.full((128, 128), 1.0 / D, BFNP),
    }

    nc = _get_program(S)
    in_maps = []
    for i in range(NCORES):
        m = dict(shared)
        shard = out_W[i * VS : (i + 1) * VS]
        m["outw_t"] = np.concatenate(
            [_wT_kmajor(shard[q * 500 : (q + 1) * 500], BFNP) for q in range(8)],
            axis=1,
        )
        in_maps.append(m)

    global LAST_RESULTS
    if TRACE:
        res = run_bass_kernel_spmd(nc, in_maps, list(range(NCORES)), trace=True)
    else:
        res = run_bass_kernel_spmd(nc, in_maps, list(range(NCORES)))
    LAST_RESULTS = res
    shards = []
    for i in range(NCORES):
        lg = res.results[i]["logits"].astype(np.float32).reshape(S, B, VS)
        shards.append(lg.transpose(1, 0, 2))
    logits = np.concatenate(shards, axis=2)  # (B, S, V)
    logits = logits + out_b[None, None, :]
    return logits.astype(np.float32)


if __name__ == "__main__":
    pass


# revision 31
# speedup vs baseline: 1.0140x; 1.0140x over previous
"""Trainium2 Bass kernel for nn_BreakthroughSNN (predictive-coding SNN LM).

Strategy (v2):
  - Replicate the sequential 128-step SNN recurrence on all 8 cores (per-step
    tensors are tiny: B=16, D=512); shard the vocab dim of the output
    projection (V=32000 -> 4000 per core). No collectives.
  - All per-step activations live in "T-layout" SBUF tiles [128, 64]:
    partition p, free index c*16+b  <->  element (d = c*128+p, b).
  - All matmuls run WEIGHT-STATIONARY in bf16: out.T chunks [128(dout),16(b)]
    accumulate over 4 k-chunks with the weight [128,128] chunk as the
    stationary operand and the (transposed) activation [128,16] as the
    moving operand.  Outputs land directly in T-layout PSUM -- no
    N-layout intermediates, no PE transposes, no fp32 double-pass.
  - err = LN(relu(bu - pred)) feeding err @ inf_W.T is folded:
      err@W.T = istd * ((r*g)@W.T) - mu*istd*(g@W.T) + (b@W.T + inf_b)
    with r = relu(bu - pred); g folded into the weights on the host;
    G2 = g@W.T and B2 = b@W.T + inf_b host-precomputed.  LN1 stats run
    concurrently with the matmul.
  - Top-down: bu=0 => err = ne_b exactly, so the TD inf-matmul input is the
    constant B2; only gen matmuls remain in the TD chain.  The j=L-1 TD gen
    matmul (td = states[L-1]) doubles as next step's xgen[L-1].  For j<L-1
    the TD matmul and the xgen matmul share one weight pass via an
    interleaved moving tile [128, (k,src,b)].
  - relu(relu(s) - p) = relu(s - p) for p in {0,1} removes the bottom-up relu.
  - Layer-stacked LIF updates: the three mem_gen decay/spike/reset updates at
    step start, the three mem_inf "base" updates, and the three TD mem_inf
    updates each run as single [128, 192] DVE ops.
  - LayerNorm partition-reductions via a single PE pass with a FULL [128,128]
    ones/D stationary: every partition receives the same (mean | E v^2) sums,
    so no separate partition-broadcast matmul is needed and the whole
    stats tail (mu^2 / var / 1/sqrt / mu*istd) runs as wide [128,16] ops.
    1/sqrt(var+eps) is one Abs_reciprocal_sqrt activation (LUT, ~4e-5 rel).
  - The spike+state-add (su -> states+su) is fused into one
    scalar_tensor_tensor; resets are (mem<1)*mem scalar_tensor_tensor ops.
"""

import sys

sys.path.insert(0, "/opt/trn_rl_repo")

import numpy as np
import ml_dtypes

import concourse.bass as bass
import concourse.bacc as bacc
from concourse import mybir
from concourse.bass_utils import run_bass_kernel_spmd
from concourse.tile import TileContext

F32 = mybir.dt.float32
BF16 = mybir.dt.bfloat16
Alu = mybir.AluOpType
Act = mybir.ActivationFunctionType
BFNP = ml_dtypes.bfloat16

B, S, V, D, L = 16, 128, 32000, 512, 3
NCORES = 8
VS = V // NCORES  # 4000 vocab rows per core
DECAY = float(np.exp(-1.0 / 2.0))
THR = 1.0
EPS = 1e-5
C = D // 128  # 4 d-chunks
W64 = C * B  # 64: free width of a T-layout tile
STK = L * W64  # 192: layer-stacked tile width


def _wT_kmajor(W, dt=np.float32):
    """W: (D_out, D_in) -> SBUF layout [128, C*D_out], k-chunk-major.

    slice [:, k*D_out:(k+1)*D_out] is W.T[k*128:(k+1)*128, :]."""
    Dout, Din = W.shape
    return (
        np.ascontiguousarray(W.T)
        .reshape(Din // 128, 128, Dout)
        .transpose(1, 0, 2)
        .reshape(128, (Din // 128) * Dout)
        .astype(dt)
    )


def _vec_T(v):
    """v: (D,) -> T-layout tile [128, 64] (b-independent broadcast)."""
    t = v.reshape(C, 128).T  # [128, C]
    return np.ascontiguousarray(
        np.broadcast_to(t[:, :, None], (128, C, B)).reshape(128, W64)
    ).astype(np.float32)


def _tok_T(tok, dt):
    """tok: (B,S,D) -> c-major [128, C*S*16]: [p, c*S*16 + t*16 + b]."""
    a = tok.transpose(2, 1, 0)  # [D, S, B]
    a = a.reshape(C, 128, S, B).transpose(1, 0, 2, 3)  # [128, C, S, B]
    return np.ascontiguousarray(a.reshape(128, C * S * B)).astype(dt)


def _build(nS):
    """Build the Bass program for nS recurrence steps."""
    nc = bacc.Bacc(None, target_bir_lowering=False)

    # ---- DRAM parameters ----
    d_tok = nc.declare_dram_parameter("tok_t", [128, nS * W64], BF16, isOutput=False)
    d_encw = nc.declare_dram_parameter("encw_t", [128, C * D], BF16, isOutput=False)
    d_genw = nc.declare_dram_parameter("genw_t", [L, 128, C * D], BF16, isOutput=False)
    d_infw = nc.declare_dram_parameter("infw_t", [L, 128, C * D], BF16, isOutput=False)
    d_outw = nc.declare_dram_parameter("outw_t", [128, C * VS], BF16, isOutput=False)
    # small constants, stacked T-layout tiles along free dim [128, 15*64]:
    # 0..2 genb_T, 3..5 B2_T, 6..8 G2_T, 9..11 nsg_T, 12..14 nsb_T
    d_ctile = nc.declare_dram_parameter("ctiles", [128, 15 * W64], F32, isOutput=False)
    d_encb = nc.declare_dram_parameter("encb_c", [128, C], F32, isOutput=False)
    d_onesb = nc.declare_dram_parameter("onesb", [128, 128], BF16, isOutput=False)
    d_logits = nc.declare_dram_parameter("logits", [nS * B, VS], BF16, isOutput=True)

    with TileContext(nc) as tc:
        with (
            tc.tile_pool(name="const", bufs=1) as cpool,
            tc.tile_pool(name="state", bufs=1) as spool,
            tc.tile_pool(name="work", bufs=3) as wpool,
            tc.tile_pool(name="stat", bufs=4) as stpool,
            tc.tile_pool(name="fin", bufs=4) as fpool,
            tc.tile_pool(name="owq", bufs=2) as owq,
            tc.tile_pool(name="psP", bufs=2, space="PSUM") as psP,
            tc.tile_pool(name="psM", bufs=3, space="PSUM") as psM,
            tc.tile_pool(name="psS", bufs=3, space="PSUM") as psS,
        ):
            # ---- load constants / weights into SBUF ----
            encw = cpool.tile([128, C * D], BF16, name="encw")
            nc.sync.dma_start(out=encw, in_=d_encw[:, :])
            genw = [cpool.tile([128, C * D], BF16, tag=f"genw{j}", name=f"genw{j}") for j in range(L)]
            infw = [cpool.tile([128, C * D], BF16, tag=f"infw{j}", name=f"infw{j}") for j in range(L)]
            for j in range(L):
                nc.sync.dma_start(out=genw[j], in_=d_genw[j])
                nc.sync.dma_start(out=infw[j], in_=d_infw[j])

            ct = cpool.tile([128, 15 * W64], F32, name="ct")
            nc.sync.dma_start(out=ct, in_=d_ctile[:, :])
            _cs = lambda i: ct[:, i * W64 : (i + 1) * W64]
            genbS = ct[:, 0:STK]  # stacked genb
            genbT = [_cs(j) for j in range(3)]
            B2S = ct[:, STK : 2 * STK]  # stacked B2
            G2T = [_cs(6 + j) for j in range(3)]
            nsgT = [_cs(9 + j) for j in range(3)]
            nsbT = [_cs(12 + j) for j in range(3)]

            encb = cpool.tile([128, C], F32, name="encb")
            nc.sync.dma_start(out=encb, in_=d_encb[:, :])

            onesb = cpool.tile([128, 128], BF16, name="onesb")
            nc.sync.dma_start(out=onesb, in_=d_onesb[:, :])

            epst = cpool.tile([128, 1], F32, name="epst")
            nc.vector.memset(epst, EPS)

            # tok tile doubles as td history (tok fully consumed in prologue)
            toktd = cpool.tile([128, nS * W64], BF16, name="toktd")
            nc.sync.dma_start(out=toktd, in_=d_tok[:, :])
            xenc = cpool.tile([128, nS * W64], F32, name="xenc")

            # ---- persistent state tiles ----
            mem_enc = spool.tile([128, W64], F32, tag="mem_enc", name="mem_enc")
            mem_gen = spool.tile([128, STK], F32, tag="mgen", name="mgen")
            mem_inf = spool.tile([128, STK], F32, tag="minf", name="minf")
            states = spool.tile([128, STK], BF16, tag="states", name="states")
            xgen = spool.tile([128, STK], F32, tag="xgen", name="xgen")
            # interleaved (k, src, b) moving tiles doubling as storage for
            # states[0] / states[1] (src=1) and pred_td[1] / pred_td[2] (src=0)
            mvt = [spool.tile([128, 2 * W64], BF16, tag=f"mvt{j}", name=f"mvt{j}")
                   for j in range(2)]
            mvt4 = [m.rearrange("p (k s b) -> p k s b", s=2, b=B) for m in mvt]

            def stview(j):
                """[128, C, B] view of states[j]."""
                if j == L - 1:
                    return states[:, j * W64 : (j + 1) * W64].rearrange(
                        "p (c b) -> p c b", c=C
                    )
                return mvt4[j][:, :, 1, :]

            nc.vector.memset(mem_enc, 0.0)
            nc.vector.memset(mem_gen, 0.0)
            nc.vector.memset(mem_inf, 0.0)
            nc.vector.memset(states, 0.0)
            nc.vector.memset(mvt[0], 0.0)
            nc.vector.memset(mvt[1], 0.0)
            # states0 = 0 -> x_gen(t=0) = gen_b
            nc.vector.tensor_copy(xgen, genbS)

            # PE pre-touch so the first real matmul carries only one
            # DMA-queue wait.
            ptch = psS.tile([1, 1], F32, tag="ps", name="ptch")
            nc.tensor.matmul(ptch, onesb[0:1, 0:1], onesb[0:1, 0:1], start=True, stop=True)

            # ---- prologue: x_enc = tok @ enc_W.T + enc_b, all steps ----
            TB = nS * B  # cols per c-chunk in c-major tok layout
            xenc4 = xenc.rearrange("p (t c b) -> p c t b", c=C, b=B)
            nblk = 4 if nS >= 4 else nS
            tblk = nS // nblk  # steps per block
            fd = tblk * B  # 512 moving cols per block
            for m in range(C):
                for ti in range(nblk):
                    pE = psP.tile([128, 512], F32, tag="pbig", name="pE")
                    for k in range(C):
                        nc.tensor.matmul(
                            pE[:, :fd],
                            encw[:, k * D + m * 128 : k * D + m * 128 + 128],
                            toktd[:, k * TB + ti * fd : k * TB + (ti + 1) * fd],
                            start=(k == 0),
                            stop=(k == C - 1),
                        )
                    nc.vector.tensor_scalar(
                        out=xenc4[:, m, ti * tblk : (ti + 1) * tblk, :],
                        in0=pE[:, :fd].rearrange("p (t b) -> p t b", b=B),
                        scalar1=encb[:, m : m + 1],
                        scalar2=None,
                        op0=Alu.add,
                    )

            # ---- helper closures ----
            def as3(t):
                return t.rearrange("p (c b) -> p c b", c=C)

            def bc(bsrc, lo):
                """[128,16] slice of bsrc broadcast to [128,4,16]."""
                return bsrc[:, None, lo : lo + 16].broadcast_to([128, C, 16])

            def ln_stats(vtile, tag):
                """vtile: [128,128] bf16 with v in cols 0:64 and v^2 in 64:128.
                Returns bsrc [128,32] SBUF: cols 0:16 istd, 16:32 mu*istd,
                identical on every partition (full-ones stationary broadcasts
                the sums to all partitions -- no separate bcast matmul)."""
                pst = psS.tile([128, 32], F32, tag="ps", name="pst")
                v4 = vtile.rearrange("p (g c b) -> p c g b", g=2, c=C)
                for c in range(C):
                    nc.tensor.matmul(
                        pst, onesb, v4[:, c], start=(c == 0), stop=(c == C - 1)
                    )
                mu2 = stpool.tile([128, 16], F32, tag=f"mu2_{tag}", name="mu2")
                nc.scalar.activation(out=mu2, in_=pst[:, 0:16], func=Act.Square)
                bsrc = wpool.tile([128, 32], F32, tag=f"bsrc_{tag}", name="bsrc")
                var = bsrc[:, 16:32]
                nc.vector.tensor_sub(var, pst[:, 16:32], mu2)
                nc.scalar.activation(
                    out=bsrc[:, 0:16], in_=var, func=Act.Abs_reciprocal_sqrt,
                    bias=epst[:, 0:1],
                )
                nc.vector.tensor_mul(bsrc[:, 16:32], pst[:, 0:16], bsrc[:, 0:16])
                return bsrc

            def mm_ws(psum, w_sb, rhs_of_k, ncols):
                """Weight-stationary matmul: psum[:, m*ncols:(m+1)*ncols]
                accumulates W_chunk.T-stationary passes; rhs_of_k(k) gives the
                [128, ncols] moving slice for k-chunk k."""
                for m in range(C):
                    for k in range(C):
                        nc.tensor.matmul(
                            psum[:, m * ncols : (m + 1) * ncols],
                            w_sb[:, k * D + m * 128 : k * D + m * 128 + 128],
                            rhs_of_k(k),
                            start=(k == 0),
                            stop=(k == C - 1),
                        )

            # ---- main recurrence ----
            for t in range(nS):
                # encoder LIF
                nc.vector.scalar_tensor_tensor(
                    out=mem_enc, in0=mem_enc, scalar=DECAY,
                    in1=xenc[:, t * W64 : (t + 1) * W64],
                    op0=Alu.mult, op1=Alu.add,
                )
                bu0 = wpool.tile([128, W64], BF16, tag="bu0", name="bu0")
                nc.vector.tensor_scalar(
                    out=bu0, in0=mem_enc, scalar1=THR, scalar2=None, op0=Alu.is_ge
                )
                nc.vector.scalar_tensor_tensor(
                    out=mem_enc, in0=mem_enc, scalar=THR, in1=mem_enc,
                    op0=Alu.is_lt, op1=Alu.mult,
                )

                # stacked gen LIF: pred_j for all layers at once
                nc.vector.scalar_tensor_tensor(
                    out=mem_gen, in0=mem_gen, scalar=DECAY, in1=xgen,
                    op0=Alu.mult, op1=Alu.add,
                )
                pred = wpool.tile([128, STK], BF16, tag="pred", name="pred")
                nc.vector.tensor_scalar(
                    out=pred, in0=mem_gen, scalar1=THR, scalar2=None, op0=Alu.is_ge
                )
                nc.vector.scalar_tensor_tensor(
                    out=mem_gen, in0=mem_gen, scalar=THR, in1=mem_gen,
                    op0=Alu.is_lt, op1=Alu.mult,
                )
                # stacked decayed mem_inf base: base = mem_inf*dec + B2
                basei = wpool.tile([128, STK], F32, tag="basei", name="basei")
                nc.vector.scalar_tensor_tensor(
                    out=basei, in0=mem_inf, scalar=DECAY, in1=B2S,
                    op0=Alu.mult, op1=Alu.add,
                )
                # stacked TD-phase gen pre-bias: mgp = mem_gen*dec + gen_b
                mgp = wpool.tile([128, STK], F32, tag="mgp", name="mgp")
                nc.vector.scalar_tensor_tensor(
                    out=mgp, in0=mem_gen, scalar=DECAY, in1=genbS,
                    op0=Alu.mult, op1=Alu.add,
                )
                # thr2 = THR - mgp: lets the TD spike fire straight off the
                # gen-matmul PSUM (pG >= thr2), keeping the mem_gen update
                # off the td critical path
                thr2 = wpool.tile([128, STK], F32, tag="thr2", name="thr2")
                nc.vector.tensor_scalar(
                    out=thr2, in0=mgp, scalar1=-1.0, scalar2=THR,
                    op0=Alu.mult, op1=Alu.add,
                )

                # ---- bottom-up ----
                for j in range(L):
                    jc = slice(j * W64, (j + 1) * W64)
                    # r = relu(bu - pred)  (bu = bu0 or states[j-1]; exact for
                    # j>0 because relu(relu(s)-p) = relu(s-p) for p in {0,1})
                    bu_in = as3(bu0) if j == 0 else stview(j - 1)
                    r2 = wpool.tile([128, 2 * W64], BF16, tag="r2", name="r2")
                    r = r2[:, 0:W64]
                    nc.vector.tensor_tensor(
                        out=as3(r), in0=bu_in, in1=as3(pred[:, jc]), op=Alu.subtract
                    )
                    nc.vector.tensor_scalar(
                        out=r, in0=r, scalar1=0.0, scalar2=None, op0=Alu.max
                    )
                    # square on DVE: stays in-queue behind the relu (no ACT
                    # handoff on the critical path)
                    nc.vector.tensor_mul(r2[:, W64 : 2 * W64], r, r)
                    # stats matmuls first on the PE queue (their tail chain is
                    # the critical path; pI is consumed much later)
                    bs1 = ln_stats(r2, f"l1_{j}")
                    pI = psM.tile([128, W64], F32, tag="pmm", name="pI")
                    mm_ws(pI, infw[j], lambda k: r2[:, k * B : (k + 1) * B], B)

                    # mem_inf' = base - mu*istd*G2 + istd*(folded matmul)
                    gsc = wpool.tile([128, W64], F32, tag="gsc", name="gsc")
                    nc.vector.tensor_mul(as3(gsc), as3(G2T[j]), bc(bs1, 16))
                    nc.vector.tensor_sub(gsc, basei[:, jc], gsc)
                    xsc = wpool.tile([128, W64], F32, tag="xsc", name="xsc")
                    nc.vector.tensor_mul(as3(xsc), as3(pI), bc(bs1, 0))
                    nc.vector.tensor_add(mem_inf[:, jc], xsc, gsc)

                    # state' = LN_ns(state + su); su = spike(mem_inf') fused in
                    w2 = wpool.tile([128, 2 * W64], BF16, tag="w2", name="w2")
                    wv = w2[:, 0:W64]
                    nc.vector.scalar_tensor_tensor(
                        out=as3(wv), in0=as3(mem_inf[:, jc]), scalar=THR,
                        in1=stview(j), op0=Alu.is_ge, op1=Alu.add,
                    )
                    nc.vector.scalar_tensor_tensor(
                        out=mem_inf[:, jc], in0=mem_inf[:, jc], scalar=THR,
                        in1=mem_inf[:, jc], op0=Alu.is_lt, op1=Alu.mult,
                    )
                    nc.vector.tensor_mul(w2[:, W64 : 2 * W64], wv, wv)
                    bs2 = ln_stats(w2, f"l2_{j}")
                    t1 = wpool.tile([128, W64], F32, tag="t1", name="t1")
                    nc.vector.tensor_mul(as3(t1), as3(wv), bc(bs2, 0))
                    nc.vector.tensor_tensor(
                        out=as3(t1), in0=as3(t1), in1=bc(bs2, 16), op=Alu.subtract
                    )
                    nc.vector.tensor_mul(t1, t1, nsgT[j])
                    nc.vector.tensor_tensor(
                        out=stview(j), in0=as3(t1), in1=as3(nsbT[j]), op=Alu.add
                    )

                # ---- top-down ----
                # j = L-1: td = states[L-1]; this matmul doubles as xgen[L-1]
                for j in reversed(range(L)):
                    jc = slice(j * W64, (j + 1) * W64)
                    mg_pre = mgp[:, jc]
                    if j == L - 1:
                        pG = psM.tile([128, W64], F32, tag="pmm", name="pG")
                        mm_ws(pG, genw[j],
                              lambda k: states[:, j * W64 + k * B : j * W64 + (k + 1) * B],
                              B)
                        pG_td = as3(pG)      # [128, m, 16] view for mem_gen
                        pG_xg = as3(pG)
                    else:
                        mv = mvt[j]
                        pG = psM.tile([128, 2 * W64], F32, tag="pmm", name="pG2")
                        mm_ws(pG, genw[j],
                              lambda k: mv[:, 2 * B * k : 2 * B * (k + 1)], 2 * B)
                        pg4 = pG.rearrange("p (m s b) -> p m s b", s=2, b=B)
                        pG_td = pg4[:, :, 0, :]
                        pG_xg = pg4[:, :, 1, :]
                    # pred_td spike straight off the PSUM: pG >= THR - mgp
                    thr2j = thr2.rearrange("p (l c b) -> p l c b", l=L, b=B)[:, j]
                    if j > 0:
                        # write pred_td[j] into interleaved moving tile for j-1
                        nc.vector.tensor_tensor(
                            out=mvt4[j - 1][:, :, 0, :], in0=pG_td, in1=thr2j,
                            op=Alu.is_ge,
                        )
                    else:
                        # write td_t (binary) into c-major td history
                        p_out = toktd.rearrange(
                            "p (c t b) -> p c t b", c=C, b=B
                        )[:, :, t, :]
                        nc.vector.tensor_tensor(
                            out=p_out, in0=pG_td, in1=thr2j, op=Alu.is_ge,
                        )
                    # mem_gen update + reset (off the td critical path)
                    nc.vector.tensor_tensor(
                        out=as3(mem_gen[:, jc]), in0=as3(mg_pre), in1=pG_td,
                        op=Alu.add,
                    )
                    nc.vector.scalar_tensor_tensor(
                        out=mem_gen[:, jc], in0=mem_gen[:, jc], scalar=THR,
                        in1=mem_gen[:, jc], op0=Alu.is_lt, op1=Alu.mult,
                    )
                    # xgen for next step (+bias)
                    nc.vector.tensor_tensor(
                        out=as3(xgen[:, jc]), in0=pG_xg, in1=as3(genbT[j]),
                        op=Alu.add,
                    )

                # stacked TD mem_inf update with constant input B2 (spike unused)
                nc.vector.scalar_tensor_tensor(
                    out=mem_inf, in0=mem_inf, scalar=DECAY, in1=B2S,
                    op0=Alu.mult, op1=Alu.add,
                )
                nc.vector.scalar_tensor_tensor(
                    out=mem_inf, in0=mem_inf, scalar=THR, in1=mem_inf,
                    op0=Alu.is_lt, op1=Alu.mult,
                )

            # ---- final projection: logits = td @ out_W_shard.T ----
            n_mb = (nS * B) // 128 if nS * B >= 128 else 1
            t_mb = nS // n_mb  # steps per M-block
            NB = VS // 500  # 8 chunks of 500
            for nb in range(NB):
                outwq = owq.tile([128, C * 500], BF16, tag="outwq", name="outwq")
                nc.sync.dma_start(
                    out=outwq, in_=d_outw[:, nb * C * 500 : (nb + 1) * C * 500]
                )
                for mb in range(n_mb):
                    pf = psP.tile([128, 512], F32, tag="pbig", name="pf")
                    fdp = t_mb * B
                    for k in range(C):
                        nc.tensor.matmul(
                            pf[:fdp, 0:500],
                            toktd[:, k * TB + mb * fdp : k * TB + (mb + 1) * fdp],
                            outwq[:, k * 500 : (k + 1) * 500],
                            start=(k == 0),
                            stop=(k == C - 1),
                        )
                    fo = fpool.tile([128, 500], BF16, tag="fo", name="fo")
                    if mb % 2 == 0:
                        nc.scalar.copy(fo[:fdp], pf[:fdp, 0:500])
                    else:
                        nc.vector.tensor_copy(fo[:fdp], pf[:fdp, 0:500])
                    nc.sync.dma_start(
                        out=d_logits[
                            mb * fdp : (mb + 1) * fdp, nb * 500 : (nb + 1) * 500
                        ],
                        in_=fo[:fdp],
                    )

    return nc


_CACHE = {}
TRACE = False
LAST_RESULTS = None


def _get_program(nS):
    if nS not in _CACHE:
        nc = _build(nS)
        nc.finalize()
        _CACHE[nS] = nc
    return _CACHE[nS]


def kernel(**inputs):
    input_ids = np.asarray(inputs["input_ids"])
    emb = np.asarray(inputs["emb"], np.float32)
    enc_W = np.asarray(inputs["enc_W"], np.float32)
    enc_b = np.asarray(inputs["enc_b"], np.float32)
    gen_W = np.asarray(inputs["gen_W"], np.float32)
    gen_b = np.asarray(inputs["gen_b"], np.float32)
    inf_W = np.asarray(inputs["inf_W"], np.float32)
    inf_b = np.asarray(inputs["inf_b"], np.float32)
    ns_g = np.asarray(inputs["ns_g"], np.float32)
    ns_b = np.asarray(inputs["ns_b"], np.float32)
    ne_g = np.asarray(inputs["ne_g"], np.float32)
    ne_b = np.asarray(inputs["ne_b"], np.float32)
    out_W = np.asarray(inputs["out_W"], np.float32)
    out_b = np.asarray(inputs["out_b"], np.float32)

    nB, nS = input_ids.shape
    assert (nB, nS) == (B, S), (nB, nS)

    tok = emb[input_ids]  # (B, S, D) host gather

    # host-side constant folding
    ctiles = np.zeros((15, 128, W64), np.float32)
    genw_t = np.zeros((L, 128, C * D), BFNP)
    infw_t = np.zeros((L, 128, C * D), BFNP)
    for j in range(L):
        ctiles[j] = _vec_T(gen_b[j])
        B2 = ne_b[j] @ inf_W[j].T + inf_b[j]
        G2 = ne_g[j] @ inf_W[j].T
        ctiles[3 + j] = _vec_T(B2)
        ctiles[6 + j] = _vec_T(G2)
        ctiles[9 + j] = _vec_T(ns_g[j])
        ctiles[12 + j] = _vec_T(ns_b[j])
        genw_t[j] = _wT_kmajor(gen_W[j], BFNP)
        infw_t[j] = _wT_kmajor(inf_W[j] * ne_g[j][None, :], BFNP)
    ctiles_packed = np.ascontiguousarray(
        ctiles.transpose(1, 0, 2).reshape(128, 15 * W64)
    )

    shared = {
        "tok_t": _tok_T(tok, BFNP),
        "encw_t": _wT_kmajor(enc_W, BFNP),
        "genw_t": genw_t,
        "infw_t": infw_t,
        "ctiles": ctiles_packed,
        "encb_c": np.ascontiguousarray(enc_b.reshape(C, 128).T).astype(np.float32),
        "onesb": np.full((128, 128), 1.0 / D, BFNP),
    }

    nc = _get_program(S)
    in_maps = []
    for i in range(NCORES):
        m = dict(shared)
        shard = out_W[i * VS : (i + 1) * VS]
        m["outw_t"] = np.concatenate(
            [_wT_kmajor(shard[q * 500 : (q + 1) * 500], BFNP) for q in range(8)],
            axis=1,
        )
        in_maps.append(m)

    global LAST_RESULTS
    if TRACE:
        res = run_bass_kernel_spmd(nc, in_maps, list(range(NCORES)), trace=True)
    else:
        res = run_bass_kernel_spmd(nc, in_maps, list(range(NCORES)))
    LAST_RESULTS = res
    shards = []
    for i in range(NCORES):
        lg = res.results[i]["logits"].astype(np.float32).reshape(S, B, VS)
        shards.append(lg.transpose(1, 0, 2))
    logits = np.concatenate(shards, axis=2)  # (B, S, V)
    logits = logits + out_b[None, None, :]
    return logits.astype(np.float32)


if __name__ == "__main__":
    pass


# revision 32
# speedup vs baseline: 1.0144x; 1.0004x over previous
"""Trainium2 Bass kernel for nn_BreakthroughSNN (predictive-coding SNN LM).

Strategy (v2):
  - Replicate the sequential 128-step SNN recurrence on all 8 cores (per-step
    tensors are tiny: B=16, D=512); shard the vocab dim of the output
    projection (V=32000 -> 4000 per core). No collectives.
  - All per-step activations live in "T-layout" SBUF tiles [128, 64]:
    partition p, free index c*16+b  <->  element (d = c*128+p, b).
  - All matmuls run WEIGHT-STATIONARY in bf16: out.T chunks [128(dout),16(b)]
    accumulate over 4 k-chunks with the weight [128,128] chunk as the
    stationary operand and the (transposed) activation [128,16] as the
    moving operand.  Outputs land directly in T-layout PSUM -- no
    N-layout intermediates, no PE transposes, no fp32 double-pass.
  - err = LN(relu(bu - pred)) feeding err @ inf_W.T is folded:
      err@W.T = istd * ((r*g)@W.T) - mu*istd*(g@W.T) + (b@W.T + inf_b)
    with r = relu(bu - pred); g folded into the weights on the host;
    G2 = g@W.T and B2 = b@W.T + inf_b host-precomputed.  LN1 stats run
    concurrently with the matmul.
  - Top-down: bu=0 => err = ne_b exactly, so the TD inf-matmul input is the
    constant B2; only gen matmuls remain in the TD chain.  The j=L-1 TD gen
    matmul (td = states[L-1]) doubles as next step's xgen[L-1].  For j<L-1
    the TD matmul and the xgen matmul share one weight pass via an
    interleaved moving tile [128, (k,src,b)].
  - relu(relu(s) - p) = relu(s - p) for p in {0,1} removes the bottom-up relu.
  - Layer-stacked LIF updates: the three mem_gen decay/spike/reset updates at
    step start, the three mem_inf "base" updates, and the three TD mem_inf
    updates each run as single [128, 192] DVE ops.
  - LayerNorm partition-reductions via a single PE pass with a FULL [128,128]
    ones/D stationary: every partition receives the same (mean | E v^2) sums,
    so no separate partition-broadcast matmul is needed and the whole
    stats tail (mu^2 / var / 1/sqrt / mu*istd) runs as wide [128,16] ops.
    1/sqrt(var+eps) is one Abs_reciprocal_sqrt activation (LUT, ~4e-5 rel).
  - The spike+state-add (su -> states+su) is fused into one
    scalar_tensor_tensor; resets are (mem<1)*mem scalar_tensor_tensor ops.
"""

import sys

sys.path.insert(0, "/opt/trn_rl_repo")

import numpy as np
import ml_dtypes

import concourse.bass as bass
import concourse.bacc as bacc
from concourse import mybir
from concourse.bass_utils import run_bass_kernel_spmd
from concourse.tile import TileContext

F32 = mybir.dt.float32
BF16 = mybir.dt.bfloat16
Alu = mybir.AluOpType
Act = mybir.ActivationFunctionType
BFNP = ml_dtypes.bfloat16

B, S, V, D, L = 16, 128, 32000, 512, 3
NCORES = 8
VS = V // NCORES  # 4000 vocab rows per core
DECAY = float(np.exp(-1.0 / 2.0))
THR = 1.0
EPS = 1e-5
C = D // 128  # 4 d-chunks
W64 = C * B  # 64: free width of a T-layout tile
STK = L * W64  # 192: layer-stacked tile width


def _wT_kmajor(W, dt=np.float32):
    """W: (D_out, D_in) -> SBUF layout [128, C*D_out], k-chunk-major.

    slice [:, k*D_out:(k+1)*D_out] is W.T[k*128:(k+1)*128, :]."""
    Dout, Din = W.shape
    return (
        np.ascontiguousarray(W.T)
        .reshape(Din // 128, 128, Dout)
        .transpose(1, 0, 2)
        .reshape(128, (Din // 128) * Dout)
        .astype(dt)
    )


def _vec_T(v):
    """v: (D,) -> T-layout tile [128, 64] (b-independent broadcast)."""
    t = v.reshape(C, 128).T  # [128, C]
    return np.ascontiguousarray(
        np.broadcast_to(t[:, :, None], (128, C, B)).reshape(128, W64)
    ).astype(np.float32)


def _tok_T(tok, dt):
    """tok: (B,S,D) -> c-major [128, C*S*16]: [p, c*S*16 + t*16 + b]."""
    a = tok.transpose(2, 1, 0)  # [D, S, B]
    a = a.reshape(C, 128, S, B).transpose(1, 0, 2, 3)  # [128, C, S, B]
    return np.ascontiguousarray(a.reshape(128, C * S * B)).astype(dt)


def _build(nS):
    """Build the Bass program for nS recurrence steps."""
    nc = bacc.Bacc(None, target_bir_lowering=False)

    # ---- DRAM parameters ----
    d_tok = nc.declare_dram_parameter("tok_t", [128, nS * W64], BF16, isOutput=False)
    d_encw = nc.declare_dram_parameter("encw_t", [128, C * D], BF16, isOutput=False)
    d_genw = nc.declare_dram_parameter("genw_t", [L, 128, C * D], BF16, isOutput=False)
    d_infw = nc.declare_dram_parameter("infw_t", [L, 128, C * D], BF16, isOutput=False)
    d_outw = nc.declare_dram_parameter("outw_t", [128, C * VS], BF16, isOutput=False)
    # small constants, stacked T-layout tiles along free dim [128, 15*64]:
    # 0..2 genb_T, 3..5 B2_T, 6..8 G2_T, 9..11 nsg_T, 12..14 nsb_T
    d_ctile = nc.declare_dram_parameter("ctiles", [128, 15 * W64], F32, isOutput=False)
    d_encb = nc.declare_dram_parameter("encb_c", [128, C], F32, isOutput=False)
    d_onesb = nc.declare_dram_parameter("onesb", [128, 128], BF16, isOutput=False)
    d_logits = nc.declare_dram_parameter("logits", [nS * B, VS], BF16, isOutput=True)

    with TileContext(nc) as tc:
        with (
            tc.tile_pool(name="const", bufs=1) as cpool,
            tc.tile_pool(name="state", bufs=1) as spool,
            tc.tile_pool(name="work", bufs=3) as wpool,
            tc.tile_pool(name="stat", bufs=4) as stpool,
            tc.tile_pool(name="fin", bufs=4) as fpool,
            tc.tile_pool(name="owq", bufs=2) as owq,
            tc.tile_pool(name="psP", bufs=2, space="PSUM") as psP,
            tc.tile_pool(name="psM", bufs=3, space="PSUM") as psM,
            tc.tile_pool(name="psS", bufs=3, space="PSUM") as psS,
        ):
            # ---- load constants / weights into SBUF ----
            encw = cpool.tile([128, C * D], BF16, name="encw")
            nc.sync.dma_start(out=encw, in_=d_encw[:, :])
            genw = [cpool.tile([128, C * D], BF16, tag=f"genw{j}", name=f"genw{j}") for j in range(L)]
            infw = [cpool.tile([128, C * D], BF16, tag=f"infw{j}", name=f"infw{j}") for j in range(L)]
            for j in range(L):
                nc.sync.dma_start(out=genw[j], in_=d_genw[j])
                nc.sync.dma_start(out=infw[j], in_=d_infw[j])

            ct = cpool.tile([128, 15 * W64], F32, name="ct")
            nc.sync.dma_start(out=ct, in_=d_ctile[:, :])
            _cs = lambda i: ct[:, i * W64 : (i + 1) * W64]
            genbS = ct[:, 0:STK]  # stacked genb
            genbT = [_cs(j) for j in range(3)]
            B2S = ct[:, STK : 2 * STK]  # stacked B2
            G2T = [_cs(6 + j) for j in range(3)]
            nsgT = [_cs(9 + j) for j in range(3)]
            nsbT = [_cs(12 + j) for j in range(3)]

            encb = cpool.tile([128, C], F32, name="encb")
            nc.sync.dma_start(out=encb, in_=d_encb[:, :])

            onesb = cpool.tile([128, 128], BF16, name="onesb")
            nc.sync.dma_start(out=onesb, in_=d_onesb[:, :])

            epst = cpool.tile([128, 1], F32, name="epst")
            nc.vector.memset(epst, EPS)

            # tok tile doubles as td history (tok fully consumed in prologue)
            toktd = cpool.tile([128, nS * W64], BF16, name="toktd")
            nc.sync.dma_start(out=toktd, in_=d_tok[:, :])
            xenc = cpool.tile([128, nS * W64], F32, name="xenc")

            # ---- persistent state tiles ----
            mem_enc = spool.tile([128, W64], F32, tag="mem_enc", name="mem_enc")
            mem_gen = spool.tile([128, STK], F32, tag="mgen", name="mgen")
            mem_inf = spool.tile([128, STK], F32, tag="minf", name="minf")
            states = spool.tile([128, STK], BF16, tag="states", name="states")
            xgen = spool.tile([128, STK], F32, tag="xgen", name="xgen")
            # interleaved (k, src, b) moving tiles doubling as storage for
            # states[0] / states[1] (src=1) and pred_td[1] / pred_td[2] (src=0)
            mvt = [spool.tile([128, 2 * W64], BF16, tag=f"mvt{j}", name=f"mvt{j}")
                   for j in range(2)]
            mvt4 = [m.rearrange("p (k s b) -> p k s b", s=2, b=B) for m in mvt]

            def stview(j):
                """[128, C, B] view of states[j]."""
                if j == L - 1:
                    return states[:, j * W64 : (j + 1) * W64].rearrange(
                        "p (c b) -> p c b", c=C
                    )
                return mvt4[j][:, :, 1, :]

            nc.vector.memset(mem_enc, 0.0)
            nc.vector.memset(mem_gen, 0.0)
            nc.vector.memset(mem_inf, 0.0)
            nc.vector.memset(states, 0.0)
            nc.vector.memset(mvt[0], 0.0)
            nc.vector.memset(mvt[1], 0.0)
            # states0 = 0 -> x_gen(t=0) = gen_b
            nc.vector.tensor_copy(xgen, genbS)

            # PE pre-touch so the first real matmul carries only one
            # DMA-queue wait.
            ptch = psS.tile([1, 1], F32, tag="ps", name="ptch")
            nc.tensor.matmul(ptch, onesb[0:1, 0:1], onesb[0:1, 0:1], start=True, stop=True)

            # ---- prologue: x_enc = tok @ enc_W.T + enc_b, all steps ----
            TB = nS * B  # cols per c-chunk in c-major tok layout
            xenc4 = xenc.rearrange("p (t c b) -> p c t b", c=C, b=B)
            nblk = 4 if nS >= 4 else nS
            tblk = nS // nblk  # steps per block
            fd = tblk * B  # 512 moving cols per block
            for m in range(C):
                for ti in range(nblk):
                    pE = psP.tile([128, 512], F32, tag="pbig", name="pE")
                    for k in range(C):
                        nc.tensor.matmul(
                            pE[:, :fd],
                            encw[:, k * D + m * 128 : k * D + m * 128 + 128],
                            toktd[:, k * TB + ti * fd : k * TB + (ti + 1) * fd],
                            start=(k == 0),
                            stop=(k == C - 1),
                        )
                    nc.vector.tensor_scalar(
                        out=xenc4[:, m, ti * tblk : (ti + 1) * tblk, :],
                        in0=pE[:, :fd].rearrange("p (t b) -> p t b", b=B),
                        scalar1=encb[:, m : m + 1],
                        scalar2=None,
                        op0=Alu.add,
                    )

            # ---- helper closures ----
            def as3(t):
                return t.rearrange("p (c b) -> p c b", c=C)

            def bc(bsrc, lo):
                """[128,16] slice of bsrc broadcast to [128,4,16]."""
                return bsrc[:, None, lo : lo + 16].broadcast_to([128, C, 16])

            def ln_stats(vtile, tag):
                """vtile: [128,128] bf16 with v in cols 0:64 and v^2 in 64:128.
                Returns bsrc [128,32] SBUF: cols 0:16 istd, 16:32 mu*istd,
                identical on every partition (full-ones stationary broadcasts
                the sums to all partitions -- no separate bcast matmul)."""
                pst = psS.tile([128, 32], F32, tag="ps", name="pst")
                for c in range(C):
                    nc.tensor.matmul(
                        pst, onesb, vtile[:, 32 * c : 32 * (c + 1)],
                        start=(c == 0), stop=(c == C - 1),
                    )
                mu2 = stpool.tile([128, 16], F32, tag=f"mu2_{tag}", name="mu2")
                nc.scalar.activation(out=mu2, in_=pst[:, 0:16], func=Act.Square)
                bsrc = wpool.tile([128, 32], F32, tag=f"bsrc_{tag}", name="bsrc")
                var = bsrc[:, 16:32]
                nc.vector.tensor_sub(var, pst[:, 16:32], mu2)
                nc.scalar.activation(
                    out=bsrc[:, 0:16], in_=var, func=Act.Abs_reciprocal_sqrt,
                    bias=epst[:, 0:1],
                )
                nc.vector.tensor_mul(bsrc[:, 16:32], pst[:, 0:16], bsrc[:, 0:16])
                return bsrc

            def mm_ws(psum, w_sb, rhs_of_k, ncols):
                """Weight-stationary matmul: psum[:, m*ncols:(m+1)*ncols]
                accumulates W_chunk.T-stationary passes; rhs_of_k(k) gives the
                [128, ncols] moving slice for k-chunk k."""
                for m in range(C):
                    for k in range(C):
                        nc.tensor.matmul(
                            psum[:, m * ncols : (m + 1) * ncols],
                            w_sb[:, k * D + m * 128 : k * D + m * 128 + 128],
                            rhs_of_k(k),
                            start=(k == 0),
                            stop=(k == C - 1),
                        )

            # ---- main recurrence ----
            for t in range(nS):
                # encoder LIF
                nc.vector.scalar_tensor_tensor(
                    out=mem_enc, in0=mem_enc, scalar=DECAY,
                    in1=xenc[:, t * W64 : (t + 1) * W64],
                    op0=Alu.mult, op1=Alu.add,
                )
                bu0 = wpool.tile([128, W64], BF16, tag="bu0", name="bu0")
                nc.vector.tensor_scalar(
                    out=bu0, in0=mem_enc, scalar1=THR, scalar2=None, op0=Alu.is_ge
                )
                nc.vector.scalar_tensor_tensor(
                    out=mem_enc, in0=mem_enc, scalar=THR, in1=mem_enc,
                    op0=Alu.is_lt, op1=Alu.mult,
                )

                # stacked gen LIF: pred_j for all layers at once
                nc.vector.scalar_tensor_tensor(
                    out=mem_gen, in0=mem_gen, scalar=DECAY, in1=xgen,
                    op0=Alu.mult, op1=Alu.add,
                )
                pred = wpool.tile([128, STK], BF16, tag="pred", name="pred")
                nc.vector.tensor_scalar(
                    out=pred, in0=mem_gen, scalar1=THR, scalar2=None, op0=Alu.is_ge
                )
                nc.vector.scalar_tensor_tensor(
                    out=mem_gen, in0=mem_gen, scalar=THR, in1=mem_gen,
                    op0=Alu.is_lt, op1=Alu.mult,
                )
                # stacked decayed mem_inf base: base = mem_inf*dec + B2
                basei = wpool.tile([128, STK], F32, tag="basei", name="basei")
                nc.vector.scalar_tensor_tensor(
                    out=basei, in0=mem_inf, scalar=DECAY, in1=B2S,
                    op0=Alu.mult, op1=Alu.add,
                )
                # stacked TD-phase gen pre-bias: mgp = mem_gen*dec + gen_b
                mgp = wpool.tile([128, STK], F32, tag="mgp", name="mgp")
                nc.vector.scalar_tensor_tensor(
                    out=mgp, in0=mem_gen, scalar=DECAY, in1=genbS,
                    op0=Alu.mult, op1=Alu.add,
                )
                # thr2 = THR - mgp: lets the TD spike fire straight off the
                # gen-matmul PSUM (pG >= thr2), keeping the mem_gen update
                # off the td critical path
                thr2 = wpool.tile([128, STK], F32, tag="thr2", name="thr2")
                nc.vector.tensor_scalar(
                    out=thr2, in0=mgp, scalar1=-1.0, scalar2=THR,
                    op0=Alu.mult, op1=Alu.add,
                )

                # ---- bottom-up ----
                for j in range(L):
                    jc = slice(j * W64, (j + 1) * W64)
                    # r = relu(bu - pred)  (bu = bu0 or states[j-1]; exact for
                    # j>0 because relu(relu(s)-p) = relu(s-p) for p in {0,1})
                    bu_in = as3(bu0) if j == 0 else stview(j - 1)
                    # r2 layout (c, g, b): both the 32-col stats-mm blocks and
                    # the 16-col inf-mm k-slices are contiguous moving operands
                    r2 = wpool.tile([128, 2 * W64], BF16, tag="r2", name="r2")
                    r4 = r2.rearrange("p (c g b) -> p c g b", g=2, b=B)
                    r = r4[:, :, 0, :]
                    nc.vector.tensor_tensor(
                        out=r, in0=bu_in, in1=as3(pred[:, jc]), op=Alu.subtract
                    )
                    nc.vector.tensor_scalar(
                        out=r, in0=r, scalar1=0.0, scalar2=None, op0=Alu.max
                    )
                    # square on DVE: stays in-queue behind the relu (no ACT
                    # handoff on the critical path)
                    nc.vector.tensor_tensor(
                        out=r4[:, :, 1, :], in0=r, in1=r, op=Alu.mult
                    )
                    # stats matmuls first on the PE queue (their tail chain is
                    # the critical path; pI is consumed much later)
                    bs1 = ln_stats(r2, f"l1_{j}")
                    pI = psM.tile([128, W64], F32, tag="pmm", name="pI")
                    mm_ws(pI, infw[j],
                          lambda k: r2[:, 2 * B * k : 2 * B * k + B], B)

                    # mem_inf' = base - mu*istd*G2 + istd*(folded matmul)
                    gsc = wpool.tile([128, W64], F32, tag="gsc", name="gsc")
                    nc.vector.tensor_mul(as3(gsc), as3(G2T[j]), bc(bs1, 16))
                    nc.vector.tensor_sub(gsc, basei[:, jc], gsc)
                    xsc = wpool.tile([128, W64], F32, tag="xsc", name="xsc")
                    nc.vector.tensor_mul(as3(xsc), as3(pI), bc(bs1, 0))
                    nc.vector.tensor_add(mem_inf[:, jc], xsc, gsc)

                    # state' = LN_ns(state + su); su = spike(mem_inf') fused in
                    w2 = wpool.tile([128, 2 * W64], BF16, tag="w2", name="w2")
                    w4 = w2.rearrange("p (c g b) -> p c g b", g=2, b=B)
                    wv = w4[:, :, 0, :]
                    nc.vector.scalar_tensor_tensor(
                        out=wv, in0=as3(mem_inf[:, jc]), scalar=THR,
                        in1=stview(j), op0=Alu.is_ge, op1=Alu.add,
                    )
                    nc.vector.scalar_tensor_tensor(
                        out=mem_inf[:, jc], in0=mem_inf[:, jc], scalar=THR,
                        in1=mem_inf[:, jc], op0=Alu.is_lt, op1=Alu.mult,
                    )
                    nc.vector.tensor_tensor(
                        out=w4[:, :, 1, :], in0=wv, in1=wv, op=Alu.mult
                    )
                    bs2 = ln_stats(w2, f"l2_{j}")
                    t1 = wpool.tile([128, W64], F32, tag="t1", name="t1")
                    nc.vector.tensor_mul(as3(t1), wv, bc(bs2, 0))
                    nc.vector.tensor_tensor(
                        out=as3(t1), in0=as3(t1), in1=bc(bs2, 16), op=Alu.subtract
                    )
                    nc.vector.tensor_mul(t1, t1, nsgT[j])
                    nc.vector.tensor_tensor(
                        out=stview(j), in0=as3(t1), in1=as3(nsbT[j]), op=Alu.add
                    )

                # ---- top-down ----
                # j = L-1: td = states[L-1]; this matmul doubles as xgen[L-1]
                for j in reversed(range(L)):
                    jc = slice(j * W64, (j + 1) * W64)
                    mg_pre = mgp[:, jc]
                    if j == L - 1:
                        pG = psM.tile([128, W64], F32, tag="pmm", name="pG")
                        mm_ws(pG, genw[j],
                              lambda k: states[:, j * W64 + k * B : j * W64 + (k + 1) * B],
                              B)
                        pG_td = as3(pG)      # [128, m, 16] view for mem_gen
                        pG_xg = as3(pG)
                    else:
                        mv = mvt[j]
                        pG = psM.tile([128, 2 * W64], F32, tag="pmm", name="pG2")
                        mm_ws(pG, genw[j],
                              lambda k: mv[:, 2 * B * k : 2 * B * (k + 1)], 2 * B)
                        pg4 = pG.rearrange("p (m s b) -> p m s b", s=2, b=B)
                        pG_td = pg4[:, :, 0, :]
                        pG_xg = pg4[:, :, 1, :]
                    # pred_td spike straight off the PSUM: pG >= THR - mgp
                    thr2j = thr2.rearrange("p (l c b) -> p l c b", l=L, b=B)[:, j]
                    if j > 0:
                        # write pred_td[j] into interleaved moving tile for j-1
                        nc.vector.tensor_tensor(
                            out=mvt4[j - 1][:, :, 0, :], in0=pG_td, in1=thr2j,
                            op=Alu.is_ge,
                        )
                    else:
                        # write td_t (binary) into c-major td history
                        p_out = toktd.rearrange(
                            "p (c t b) -> p c t b", c=C, b=B
                        )[:, :, t, :]
                        nc.vector.tensor_tensor(
                            out=p_out, in0=pG_td, in1=thr2j, op=Alu.is_ge,
                        )
                    # mem_gen update + reset (off the td critical path)
                    nc.vector.tensor_tensor(
                        out=as3(mem_gen[:, jc]), in0=as3(mg_pre), in1=pG_td,
                        op=Alu.add,
                    )
                    nc.vector.scalar_tensor_tensor(
                        out=mem_gen[:, jc], in0=mem_gen[:, jc], scalar=THR,
                        in1=mem_gen[:, jc], op0=Alu.is_lt, op1=Alu.mult,
                    )
                    # xgen for next step (+bias)
                    nc.vector.tensor_tensor(
                        out=as3(xgen[:, jc]), in0=pG_xg, in1=as3(genbT[j]),
                        op=Alu.add,
                    )

                # stacked TD mem_inf update with constant input B2 (spike unused)
                nc.vector.scalar_tensor_tensor(
                    out=mem_inf, in0=mem_inf, scalar=DECAY, in1=B2S,
                    op0=Alu.mult, op1=Alu.add,
                )
                nc.vector.scalar_tensor_tensor(
                    out=mem_inf, in0=mem_inf, scalar=THR, in1=mem_inf,
                    op0=Alu.is_lt, op1=Alu.mult,
                )

            # ---- final projection: logits = td @ out_W_shard.T ----
            n_mb = (nS * B) // 128 if nS * B >= 128 else 1
            t_mb = nS // n_mb  # steps per M-block
            NB = VS // 500  # 8 chunks of 500
            for nb in range(NB):
                outwq = owq.tile([128, C * 500], BF16, tag="outwq", name="outwq")
                nc.sync.dma_start(
                    out=outwq, in_=d_outw[:, nb * C * 500 : (nb + 1) * C * 500]
                )
                for mb in range(n_mb):
                    pf = psP.tile([128, 512], F32, tag="pbig", name="pf")
                    fdp = t_mb * B
                    for k in range(C):
                        nc.tensor.matmul(
                            pf[:fdp, 0:500],
                            toktd[:, k * TB + mb * fdp : k * TB + (mb + 1) * fdp],
                            outwq[:, k * 500 : (k + 1) * 500],
                            start=(k == 0),
                            stop=(k == C - 1),
                        )
                    fo = fpool.tile([128, 500], BF16, tag="fo", name="fo")
                    if mb % 2 == 0:
                        nc.scalar.copy(fo[:fdp], pf[:fdp, 0:500])
                    else:
                        nc.vector.tensor_copy(fo[:fdp], pf[:fdp, 0:500])
                    nc.sync.dma_start(
                        out=d_logits[
                            mb * fdp : (mb + 1) * fdp, nb * 500 : (nb + 1) * 500
                        ],
                        in_=fo[:fdp],
                    )

    return nc


_CACHE = {}
TRACE = False
LAST_RESULTS = None


def _get_program(nS):
    if nS not in _CACHE:
        nc = _build(nS)
        nc.finalize()
        _CACHE[nS] = nc
    return _CACHE[nS]


def kernel(**inputs):
    input_ids = np.asarray(inputs["input_ids"])
    emb = np.asarray(inputs["emb"], np.float32)
    enc_W = np.asarray(inputs["enc_W"], np.float32)
    enc_b = np.asarray(inputs["enc_b"], np.float32)
    gen_W = np.asarray(inputs["gen_W"], np.float32)
    gen_b = np.asarray(inputs["gen_b"], np.float32)
    inf_W = np.asarray(inputs["inf_W"], np.float32)
    inf_b = np.asarray(inputs["inf_b"], np.float32)
    ns_g = np.asarray(inputs["ns_g"], np.float32)
    ns_b = np.asarray(inputs["ns_b"], np.float32)
    ne_g = np.asarray(inputs["ne_g"], np.float32)
    ne_b = np.asarray(inputs["ne_b"], np.float32)
    out_W = np.asarray(inputs["out_W"], np.float32)
    out_b = np.asarray(inputs["out_b"], np.float32)

    nB, nS = input_ids.shape
    assert (nB, nS) == (B, S), (nB, nS)

    tok = emb[input_ids]  # (B, S, D) host gather

    # host-side constant folding
    ctiles = np.zeros((15, 128, W64), np.float32)
    genw_t = np.zeros((L, 128, C * D), BFNP)
    infw_t = np.zeros((L, 128, C * D), BFNP)
    for j in range(L):
        ctiles[j] = _vec_T(gen_b[j])
        B2 = ne_b[j] @ inf_W[j].T + inf_b[j]
        G2 = ne_g[j] @ inf_W[j].T
        ctiles[3 + j] = _vec_T(B2)
        ctiles[6 + j] = _vec_T(G2)
        ctiles[9 + j] = _vec_T(ns_g[j])
        ctiles[12 + j] = _vec_T(ns_b[j])
        genw_t[j] = _wT_kmajor(gen_W[j], BFNP)
        infw_t[j] = _wT_kmajor(inf_W[j] * ne_g[j][None, :], BFNP)
    ctiles_packed = np.ascontiguousarray(
        ctiles.transpose(1, 0, 2).reshape(128, 15 * W64)
    )

    shared = {
        "tok_t": _tok_T(tok, BFNP),
        "encw_t": _wT_kmajor(enc_W, BFNP),
        "genw_t": genw_t,
        "infw_t": infw_t,
        "ctiles": ctiles_packed,
        "encb_c": np.ascontiguousarray(enc_b.reshape(C, 128).T).astype(np.float32),
        "onesb": np.full((128, 128), 1.0 / D, BFNP),
    }

    nc = _get_program(S)
    in_maps = []
    for i in range(NCORES):
        m = dict(shared)
        shard = out_W[i * VS : (i + 1) * VS]
        m["outw_t"] = np.concatenate(
            [_wT_kmajor(shard[q * 500 : (q + 1) * 500], BFNP) for q in range(8)],
            axis=1,
        )
        in_maps.append(m)

    global LAST_RESULTS
    if TRACE:
        res = run_bass_kernel_spmd(nc, in_maps, list(range(NCORES)), trace=True)
    else:
        res = run_bass_kernel_spmd(nc, in_maps, list(range(NCORES)))
    LAST_RESULTS = res
    shards = []
    for i in range(NCORES):
        lg = res.results[i]["logits"].astype(np.float32).reshape(S, B, VS)
        shards.append(lg.transpose(1, 0, 2))
    logits = np.concatenate(shards, axis=2)  # (B, S, V)
    logits = logits + out_b[None, None, :]
    return logits.astype(np.float32)


if __name__ == "__main__":
    pass
